# revision 15
# baseline (speedup 1.0000x reference)
"""MoE feed-forward (1024-d model, 4096-d FFN, 8 experts, top-2) on 8 TRN2
NeuronCores.

Strategy: expert-parallel sparse routing. The gate (softmax over 8 experts +
top-2 pick) is computed on host in float64 — it is ~0.01% of the FLOPs — and
used to dispatch each token to the cores owning its two experts. Core e holds
expert e's weights and runs a dense bf16 FFN over the tokens routed to it
(padded to a static capacity), applying bias1+ReLU, bias2 and the gate
probability on device. Host scatter-adds the two per-expert contributions back
into the full [S, B, H] output. Routed compute is 2/8 of the dense reference.

Device layout (per core, token count padded to Cap, multiple of 128):
  xeT  [H=1024, Cap]        bf16  gathered tokens, transposed
  w1X  [F/512, H/128, 128, 512] bf16  expert's w1^T, swizzled so each
                                  (f-block, k-tile) DMA is contiguous
  w2X  [F/128, 128, H]      bf16  expert's w2^T, k-tile-contiguous
  b1P  [128, F/128] f32, b2P [128, H/128] f32  partition-major biases
  peb  [128, Cap]           f32   gate prob broadcast across partitions
  yT   [H, Cap]             f32   (relu(x @ w1^T + b1) @ w2^T + b2) * pe

The matmul chain stays in [feature-partition, token-free] layout: hT tiles
[128f, n] from mm1 feed mm2 as the moving operand directly, no transposes.
"""

import sys
import types

for _p in ("/opt/trn_rl_repo",):
    if _p not in sys.path:
        sys.path.insert(0, _p)

import numpy as np
import ml_dtypes

H = 1024
F = 4096
E = 8
TOPK = 2
P = 128
NCORES = 8
NCHUNK = 512  # moving-operand width per matmul (one fp32 PSUM bank)
MB1 = 512  # f-columns per w1 DMA block
WARMUP_MM = 0  # dummy matmuls to hold the PE HAM clock-gate open during DMA ramp

BF16 = ml_dtypes.bfloat16

LAST_RESULT = None  # BassKernelResults of the most recent run (for profiling)


def _ensure_ntff_hook():
    """The image's antenv package lacks axon_hooks; supply it so
    run_bass_kernel_spmd(trace=True) can capture NTFF profiles."""
    name = "antenv.axon_hooks"
    if name in sys.modules:
        return
    mod = types.ModuleType(name)
    mod._hook = None
    mod.set_axon_ntff_profile_hook = lambda h: setattr(mod, "_hook", h)
    mod.get_axon_ntff_profile_hook = lambda: mod._hook
    try:
        import antenv  # noqa: F401  real package must stay importable

        sys.modules[name] = mod
        from trn_agent_boot.trn_boot import _ntff_profile_via_ctypes

        mod._hook = _ntff_profile_via_ctypes("/opt/axon/libaxon_pjrt.so")
    except Exception:
        pass  # tracing degrades; execution is unaffected


def _build_nc(cap):
    import concourse.bacc as bacc
    import concourse.tile as tile
    from concourse import mybir

    f32 = mybir.dt.float32
    bf16 = mybir.dt.bfloat16
    Act = mybir.ActivationFunctionType
    Alu = mybir.AluOpType

    KT1 = H // P  # k-tiles of mm1 (contraction over H)
    MT1 = F // P  # m-tiles of mm1 (FFN dim on partitions)
    KT2 = F // P  # k-tiles of mm2 (contraction over F)
    HT = H // P  # m-tiles of mm2 (model dim on partitions)
    FB1 = F // MB1

    nc = bacc.Bacc(
        "TRN2",
        target_bir_lowering=False,
        debug=False,
        enable_asserts=False,
        num_devices=NCORES,
    )
    xeT = nc.dram_tensor("xeT", [H, cap], bf16, kind="ExternalInput").ap()
    w1X = nc.dram_tensor("w1X", [FB1, KT1, P, MB1], bf16, kind="ExternalInput").ap()
    b1P = nc.dram_tensor("b1P", [P, MT1], f32, kind="ExternalInput").ap()
    w2X = nc.dram_tensor("w2X", [KT2, P, H], bf16, kind="ExternalInput").ap()
    b2P = nc.dram_tensor("b2P", [P, HT], f32, kind="ExternalInput").ap()
    peb = nc.dram_tensor("peb", [P, cap], f32, kind="ExternalInput").ap()
    yT = nc.dram_tensor("yT", [H, cap], f32, kind="ExternalOutput").ap()

    # Chunk 0 is full-width: it runs while w1/w2 still stream from HBM, and
    # a wide moving operand halves the weight-bandwidth the PE demands.
    # The rest is split evenly but kept wide enough (>~150) that matmuls
    # stay above the LDWEIGHTS-rate floor (~56ns/MM).
    chunks = [(0, min(NCHUNK, cap))]
    rem = cap - chunks[0][1]
    if rem > 0:
        nck = -(-rem // NCHUNK)
        base, extra = divmod(rem, nck)
        n0 = chunks[0][1]
        for i in range(nck):
            nsz = base + (1 if i < extra else 0)
            chunks.append((n0, nsz))
            n0 += nsz

    with tile.TileContext(nc) as tc:
        with (
            tc.tile_pool(name="weights", bufs=1) as wpool,
            tc.tile_pool(name="xin", bufs=1) as xpool,
            tc.tile_pool(name="hbuf", bufs=1) as hpool,
            tc.tile_pool(name="yout", bufs=3) as ypool,
            tc.tile_pool(name="acc", bufs=8, space="PSUM") as psum_pool,
        ):
            # PE warm-up: the HAM clock gate holds the PE at 1.2 GHz until it
            # has been busy ~3.4us. Burn dummy matmuls on a memset tile while
            # the first DMAs land so real matmuls start at 2.4 GHz.
            wa = wpool.tile([P, P], bf16, tag="warm_a")
            wb = wpool.tile([P, NCHUNK], bf16, tag="warm_b")
            nc.vector.memset(wa, 0.0)
            nc.vector.memset(wb, 0.0)
            wp = psum_pool.tile([P, NCHUNK], f32, tag="ps")
            for _ in range(WARMUP_MM):
                nc.tensor.matmul(wp, wa, wb, start=True, stop=True)

            xeTr = xeT.rearrange("(kt p) c -> p kt c", p=P)
            yTr = yT.rearrange("(ht p) c -> p ht c", p=P)

            # DMA priority order (queues are FIFO): chunk-0 token slices and
            # biases, then all of w1 — the PE consumes w1 at ~210 GB/s during
            # chunk-0 mm1, so nothing else may steal its head bandwidth —
            # then the remaining token chunks, gate probs, and w2 (first
            # needed ~45us in, at mm2 of chunk 0).
            n0_0, nsz_0 = chunks[0]
            xes = xpool.tile([P, KT1, cap], bf16)
            for kt in range(KT1):
                nc.sync.dma_start(xes[:, kt, :nsz_0], xeTr[:, kt, :nsz_0])
            b1s = wpool.tile([P, MT1], f32)
            nc.sync.dma_start(b1s, b1P)
            b2s = wpool.tile([P, HT], f32)
            nc.sync.dma_start(b2s, b2P)

            # w1 first and alone (mm1 of the wide chunk 0 consumes it at
            # ~145 GB/s starting t~8us — it must own the full DMA bandwidth),
            # then w2 (first needed ~64us, finishes ~66us), then gate probs
            # and the remaining token chunks (needed ~120us).
            w1s = wpool.tile([P, KT1, F], bf16)
            for fb in range(FB1):
                for kt in range(KT1):
                    nc.sync.dma_start(
                        w1s[:, kt, fb * MB1 : (fb + 1) * MB1], w1X[fb, kt]
                    )
            w2s = wpool.tile([P, KT2, H], bf16)
            for kt in range(KT2):
                nc.sync.dma_start(w2s[:, kt, :], w2X[kt])

            pes = xpool.tile([P, cap], f32)
            nc.sync.dma_start(pes, peb)
            for kt in range(KT1):
                nc.sync.dma_start(xes[:, kt, nsz_0:], xeTr[:, kt, nsz_0:])

            for n0, nsz in chunks:
                # mm1: hT[f, t] = relu(sum_h w1T[h, f] * xeT[h, t] + b1[f])
                hs = hpool.tile([P, MT1, nsz], bf16, tag="hs")
                for mt in range(MT1):
                    ps = psum_pool.tile([P, nsz], f32, tag="ps")
                    for kt in range(KT1):
                        nc.tensor.matmul(
                            ps,
                            w1s[:, kt, mt * P : (mt + 1) * P],
                            xes[:, kt, n0 : n0 + nsz],
                            start=(kt == 0),
                            stop=(kt == KT1 - 1),
                        )
                    nc.vector.tensor_scalar(
                        hs[:, mt, :], ps, b1s[:, mt : mt + 1], 0.0, Alu.add, Alu.max
                    )

                # mm2: yT[h', t] = (sum_f w2T[f, h'] * hT[f, t] + b2[h']) * pe[t]
                for ht in range(HT):
                    ps2 = psum_pool.tile([P, nsz], f32, tag="ps")
                    for kt in range(KT2):
                        nc.tensor.matmul(
                            ps2,
                            w2s[:, kt, ht * P : (ht + 1) * P],
                            hs[:, kt, :],
                            start=(kt == 0),
                            stop=(kt == KT2 - 1),
                        )
                    ys = ypool.tile([P, nsz], f32, tag="ys")
                    nc.scalar.activation(ys, ps2, Act.Identity, bias=b2s[:, ht : ht + 1])
                    nc.vector.tensor_tensor(ys, ys, pes[:, n0 : n0 + nsz], Alu.mult)
                    nc.sync.dma_start(yTr[:, ht, n0 : n0 + nsz], ys)

    nc.compile()
    return nc


_NC_CACHE = {}


def _get_nc(cap):
    if cap not in _NC_CACHE:
        _NC_CACHE[cap] = _build_nc(cap)
    return _NC_CACHE[cap]


def _route(xf, wg):
    """Host-side gate: float64 softmax + top-2 (ties -> lower expert index,
    matching jax.lax.top_k)."""
    logits = xf.astype(np.float64) @ np.asarray(wg, np.float64).T  # [T, E]
    top2 = np.argsort(-logits, axis=1, kind="stable")[:, :TOPK]
    lmax = logits.max(axis=1, keepdims=True)
    p = np.exp(logits - lmax)
    p /= p.sum(axis=1, keepdims=True)
    topp = np.take_along_axis(p, top2, axis=1)  # [T, TOPK]
    return top2, topp


def kernel(x, wg, w1, b1, w2, b2, _trace=False):
    global LAST_RESULT
    from concourse import bass_utils

    x = np.asarray(x, np.float32)
    wg = np.asarray(wg, np.float32)
    w1 = np.asarray(w1, np.float32)
    b1 = np.asarray(b1, np.float32)
    w2 = np.asarray(w2, np.float32)
    b2 = np.asarray(b2, np.float32)

    S, B, Hd = x.shape
    assert Hd == H
    T = S * B
    xf = x.reshape(T, Hd)

    top2, topp = _route(xf, wg)

    idx = []
    gate = []
    for e in range(E):
        m = top2 == e  # [T, 2]
        tok = np.nonzero(m.any(axis=1))[0]
        idx.append(tok)
        gate.append(topp[tok][m[tok]].astype(np.float32))

    cap = max(NCHUNK, max(len(t) for t in idx))  # exact, no 128-padding needed

    KT1 = H // P
    FB1 = F // MB1
    in_maps = []
    for e in range(E):
        tok = idx[e]
        n_e = len(tok)
        xe = np.zeros((cap, Hd), np.float32)
        xe[:n_e] = xf[tok]
        pe_full = np.zeros(cap, np.float32)
        pe_full[:n_e] = gate[e]
        w1T = np.ascontiguousarray(w1[e].T).astype(BF16)  # [H, F]
        w1Xe = np.ascontiguousarray(
            w1T.reshape(KT1, P, FB1, MB1).transpose(2, 0, 1, 3)
        )
        w2T = np.ascontiguousarray(w2[e].T).astype(BF16)  # [F, H]
        in_maps.append(
            {
                "xeT": np.ascontiguousarray(xe.T).astype(BF16),
                "w1X": w1Xe,
                "b1P": np.ascontiguousarray(b1[e].reshape(F // P, P).T),
                "w2X": w2T.reshape(F // P, P, H),
                "b2P": np.ascontiguousarray(b2[e].reshape(H // P, P).T),
                "peb": np.ascontiguousarray(np.broadcast_to(pe_full, (P, cap))),
            }
        )

    if _trace:
        _ensure_ntff_hook()
        bass_utils.upload_artifacts = lambda tmpdir: f"local:{tmpdir}"

    nc = _get_nc(cap)
    res = bass_utils.run_bass_kernel_spmd(
        nc, in_maps, core_ids=list(range(NCORES)), trace=_trace
    )
    LAST_RESULT = res

    yf = np.zeros((T, Hd), np.float32)
    for e in range(E):
        tok = idx[e]
        yT_e = np.asarray(res.results[e]["yT"], np.float32)  # [H, cap]
        yf[tok] += yT_e[:, : len(tok)].T
    return yf.reshape(S, B, Hd)


# revision 16
# speedup vs baseline: 1.0024x; 1.0024x over previous
"""MoE feed-forward (1024-d model, 4096-d FFN, 8 experts, top-2) on 8 TRN2
NeuronCores.

Strategy: expert-parallel sparse routing. The gate (softmax over 8 experts +
top-2 pick) is computed on host in float64 — it is ~0.01% of the FLOPs — and
used to dispatch each token to the cores owning its two experts. Core e holds
expert e's weights and runs a dense bf16 FFN over the tokens routed to it
(padded to a static capacity), applying bias1+ReLU, bias2 and the gate
probability on device. Host scatter-adds the two per-expert contributions back
into the full [S, B, H] output. Routed compute is 2/8 of the dense reference.

Device layout (per core, token count padded to Cap, multiple of 128):
  xeT  [H=1024, Cap]        bf16  gathered tokens, transposed
  w1X  [F/512, H/128, 128, 512] bf16  expert's w1^T, swizzled so each
                                  (f-block, k-tile) DMA is contiguous
  w2X  [F/128, 128, H]      bf16  expert's w2^T, k-tile-contiguous
  b1P  [128, F/128] f32, b2P [128, H/128] f32  partition-major biases
  peb  [128, Cap]           f32   gate prob broadcast across partitions
  yT   [H, Cap]             f32   (relu(x @ w1^T + b1) @ w2^T + b2) * pe

The matmul chain stays in [feature-partition, token-free] layout: hT tiles
[128f, n] from mm1 feed mm2 as the moving operand directly, no transposes.
"""

import sys
import types

for _p in ("/opt/trn_rl_repo",):
    if _p not in sys.path:
        sys.path.insert(0, _p)

import numpy as np
import ml_dtypes

H = 1024
F = 4096
E = 8
TOPK = 2
P = 128
NCORES = 8
NCHUNK = 512  # moving-operand width per matmul (one fp32 PSUM bank)
MB1 = 512  # f-columns per w1 DMA block
WARMUP_MM = 40  # dummy matmuls to hold the PE HAM clock-gate open during DMA ramp

BF16 = ml_dtypes.bfloat16

LAST_RESULT = None  # BassKernelResults of the most recent run (for profiling)


def _ensure_ntff_hook():
    """The image's antenv package lacks axon_hooks; supply it so
    run_bass_kernel_spmd(trace=True) can capture NTFF profiles."""
    name = "antenv.axon_hooks"
    if name in sys.modules:
        return
    mod = types.ModuleType(name)
    mod._hook = None
    mod.set_axon_ntff_profile_hook = lambda h: setattr(mod, "_hook", h)
    mod.get_axon_ntff_profile_hook = lambda: mod._hook
    try:
        import antenv  # noqa: F401  real package must stay importable

        sys.modules[name] = mod
        from trn_agent_boot.trn_boot import _ntff_profile_via_ctypes

        mod._hook = _ntff_profile_via_ctypes("/opt/axon/libaxon_pjrt.so")
    except Exception:
        pass  # tracing degrades; execution is unaffected


def _build_nc(cap):
    import concourse.bacc as bacc
    import concourse.tile as tile
    from concourse import mybir

    f32 = mybir.dt.float32
    bf16 = mybir.dt.bfloat16
    Act = mybir.ActivationFunctionType
    Alu = mybir.AluOpType

    KT1 = H // P  # k-tiles of mm1 (contraction over H)
    MT1 = F // P  # m-tiles of mm1 (FFN dim on partitions)
    KT2 = F // P  # k-tiles of mm2 (contraction over F)
    HT = H // P  # m-tiles of mm2 (model dim on partitions)
    FB1 = F // MB1

    nc = bacc.Bacc(
        "TRN2",
        target_bir_lowering=False,
        debug=False,
        enable_asserts=False,
        num_devices=NCORES,
    )
    xeT = nc.dram_tensor("xeT", [H, cap], bf16, kind="ExternalInput").ap()
    w1X = nc.dram_tensor("w1X", [FB1, KT1, P, MB1], bf16, kind="ExternalInput").ap()
    b1P = nc.dram_tensor("b1P", [P, MT1], f32, kind="ExternalInput").ap()
    w2X = nc.dram_tensor("w2X", [KT2, P, H], bf16, kind="ExternalInput").ap()
    b2P = nc.dram_tensor("b2P", [P, HT], f32, kind="ExternalInput").ap()
    peb = nc.dram_tensor("peb", [P, cap], f32, kind="ExternalInput").ap()
    yT = nc.dram_tensor("yT", [H, cap], f32, kind="ExternalOutput").ap()

    # Chunk 0 is full-width: it runs while w1/w2 still stream from HBM, and
    # a wide moving operand halves the weight-bandwidth the PE demands.
    # The rest is split evenly but kept wide enough (>~150) that matmuls
    # stay above the LDWEIGHTS-rate floor (~56ns/MM).
    chunks = [(0, min(NCHUNK, cap))]
    rem = cap - chunks[0][1]
    if rem > 0:
        nck = -(-rem // NCHUNK)
        base, extra = divmod(rem, nck)
        n0 = chunks[0][1]
        for i in range(nck):
            nsz = base + (1 if i < extra else 0)
            chunks.append((n0, nsz))
            n0 += nsz

    with tile.TileContext(nc) as tc:
        with (
            tc.tile_pool(name="weights", bufs=1) as wpool,
            tc.tile_pool(name="xin", bufs=1) as xpool,
            tc.tile_pool(name="hbuf", bufs=1) as hpool,
            tc.tile_pool(name="yout", bufs=3) as ypool,
            tc.tile_pool(name="acc", bufs=8, space="PSUM") as psum_pool,
        ):
            # PE warm-up: the HAM clock gate holds the PE at 1.2 GHz until it
            # has been busy ~3.4us. Burn dummy matmuls on a memset tile while
            # the first DMAs land so real matmuls start at 2.4 GHz.
            wa = wpool.tile([P, P], bf16, tag="warm_a")
            wb = wpool.tile([P, NCHUNK], bf16, tag="warm_b")
            nc.vector.memset(wa, 0.0)
            nc.vector.memset(wb, 0.0)
            wp = psum_pool.tile([P, NCHUNK], f32, tag="ps")
            for _ in range(WARMUP_MM):
                nc.tensor.matmul(wp, wa, wb, start=True, stop=True)

            xeTr = xeT.rearrange("(kt p) c -> p kt c", p=P)
            yTr = yT.rearrange("(ht p) c -> p ht c", p=P)

            # DMA priority order (queues are FIFO): chunk-0 token slices and
            # biases, then all of w1 — the PE consumes w1 at ~210 GB/s during
            # chunk-0 mm1, so nothing else may steal its head bandwidth —
            # then the remaining token chunks, gate probs, and w2 (first
            # needed ~45us in, at mm2 of chunk 0).
            n0_0, nsz_0 = chunks[0]
            xes = xpool.tile([P, KT1, cap], bf16)
            for kt in range(KT1):
                nc.sync.dma_start(xes[:, kt, :nsz_0], xeTr[:, kt, :nsz_0])
            b1s = wpool.tile([P, MT1], f32)
            nc.sync.dma_start(b1s, b1P)
            b2s = wpool.tile([P, HT], f32)
            nc.sync.dma_start(b2s, b2P)

            # w1 first and alone (mm1 of the wide chunk 0 consumes it at
            # ~145 GB/s starting t~8us — it must own the full DMA bandwidth),
            # then w2 (first needed ~64us, finishes ~66us), then gate probs
            # and the remaining token chunks (needed ~120us).
            w1s = wpool.tile([P, KT1, F], bf16)
            for fb in range(FB1):
                for kt in range(KT1):
                    nc.sync.dma_start(
                        w1s[:, kt, fb * MB1 : (fb + 1) * MB1], w1X[fb, kt]
                    )
            w2s = wpool.tile([P, KT2, H], bf16)
            for kt in range(KT2):
                nc.sync.dma_start(w2s[:, kt, :], w2X[kt])

            pes = xpool.tile([P, cap], f32)
            nc.sync.dma_start(pes, peb)
            for kt in range(KT1):
                nc.sync.dma_start(xes[:, kt, nsz_0:], xeTr[:, kt, nsz_0:])

            for n0, nsz in chunks:
                # mm1: hT[f, t] = relu(sum_h w1T[h, f] * xeT[h, t] + b1[f])
                hs = hpool.tile([P, MT1, nsz], bf16, tag="hs")
                for mt in range(MT1):
                    ps = psum_pool.tile([P, nsz], f32, tag="ps")
                    for kt in range(KT1):
                        nc.tensor.matmul(
                            ps,
                            w1s[:, kt, mt * P : (mt + 1) * P],
                            xes[:, kt, n0 : n0 + nsz],
                            start=(kt == 0),
                            stop=(kt == KT1 - 1),
                        )
                    nc.vector.tensor_scalar(
                        hs[:, mt, :], ps, b1s[:, mt : mt + 1], 0.0, Alu.add, Alu.max
                    )

                # mm2: yT[h', t] = (sum_f w2T[f, h'] * hT[f, t] + b2[h']) * pe[t]
                for ht in range(HT):
                    ps2 = psum_pool.tile([P, nsz], f32, tag="ps")
                    for kt in range(KT2):
                        nc.tensor.matmul(
                            ps2,
                            w2s[:, kt, ht * P : (ht + 1) * P],
                            hs[:, kt, :],
                            start=(kt == 0),
                            stop=(kt == KT2 - 1),
                        )
                    ys = ypool.tile([P, nsz], f32, tag="ys")
                    nc.scalar.activation(ys, ps2, Act.Identity, bias=b2s[:, ht : ht + 1])
                    nc.vector.tensor_tensor(ys, ys, pes[:, n0 : n0 + nsz], Alu.mult)
                    nc.sync.dma_start(yTr[:, ht, n0 : n0 + nsz], ys)

    nc.compile()
    return nc


_NC_CACHE = {}


def _get_nc(cap):
    if cap not in _NC_CACHE:
        _NC_CACHE[cap] = _build_nc(cap)
    return _NC_CACHE[cap]


def _route(xf, wg):
    """Host-side gate: float64 softmax + top-2 (ties -> lower expert index,
    matching jax.lax.top_k)."""
    logits = xf.astype(np.float64) @ np.asarray(wg, np.float64).T  # [T, E]
    top2 = np.argsort(-logits, axis=1, kind="stable")[:, :TOPK]
    lmax = logits.max(axis=1, keepdims=True)
    p = np.exp(logits - lmax)
    p /= p.sum(axis=1, keepdims=True)
    topp = np.take_along_axis(p, top2, axis=1)  # [T, TOPK]
    return top2, topp


def kernel(x, wg, w1, b1, w2, b2, _trace=False):
    global LAST_RESULT
    from concourse import bass_utils

    x = np.asarray(x, np.float32)
    wg = np.asarray(wg, np.float32)
    w1 = np.asarray(w1, np.float32)
    b1 = np.asarray(b1, np.float32)
    w2 = np.asarray(w2, np.float32)
    b2 = np.asarray(b2, np.float32)

    S, B, Hd = x.shape
    assert Hd == H
    T = S * B
    xf = x.reshape(T, Hd)

    top2, topp = _route(xf, wg)

    idx = []
    gate = []
    for e in range(E):
        m = top2 == e  # [T, 2]
        tok = np.nonzero(m.any(axis=1))[0]
        idx.append(tok)
        gate.append(topp[tok][m[tok]].astype(np.float32))

    cap = max(NCHUNK, max(len(t) for t in idx))  # exact, no 128-padding needed

    KT1 = H // P
    FB1 = F // MB1
    in_maps = []
    for e in range(E):
        tok = idx[e]
        n_e = len(tok)
        xe = np.zeros((cap, Hd), np.float32)
        xe[:n_e] = xf[tok]
        pe_full = np.zeros(cap, np.float32)
        pe_full[:n_e] = gate[e]
        w1T = np.ascontiguousarray(w1[e].T).astype(BF16)  # [H, F]
        w1Xe = np.ascontiguousarray(
            w1T.reshape(KT1, P, FB1, MB1).transpose(2, 0, 1, 3)
        )
        w2T = np.ascontiguousarray(w2[e].T).astype(BF16)  # [F, H]
        in_maps.append(
            {
                "xeT": np.ascontiguousarray(xe.T).astype(BF16),
                "w1X": w1Xe,
                "b1P": np.ascontiguousarray(b1[e].reshape(F // P, P).T),
                "w2X": w2T.reshape(F // P, P, H),
                "b2P": np.ascontiguousarray(b2[e].reshape(H // P, P).T),
                "peb": np.ascontiguousarray(np.broadcast_to(pe_full, (P, cap))),
            }
        )

    if _trace:
        _ensure_ntff_hook()
        bass_utils.upload_artifacts = lambda tmpdir: f"local:{tmpdir}"

    nc = _get_nc(cap)
    res = bass_utils.run_bass_kernel_spmd(
        nc, in_maps, core_ids=list(range(NCORES)), trace=_trace
    )
    LAST_RESULT = res

    yf = np.zeros((T, Hd), np.float32)
    for e in range(E):
        tok = idx[e]
        yT_e = np.asarray(res.results[e]["yT"], np.float32)  # [H, cap]
        yf[tok] += yT_e[:, : len(tok)].T
    return yf.reshape(S, B, Hd)
